# revision 1
# baseline (speedup 1.0000x reference)
"""Cross-attention kernel for Trainium2, 8 NeuronCores, data-parallel over batch.

Computes, per batch b (one batch per core):
    q_proj = q[b] @ Wq          [Nq, E]
    k_proj = y[b] @ Wk          [Nk, E]
    v_proj = k_proj @ Wv        [Nk, F]   (faithful quirk: value() of key-projection)
    scores = q_proj @ k_proj.T / sqrt(E)
    out    = softmax(scores, -1) @ v_proj

Device-side layout strategy: all activations are kept "feature-major"
([feature_part, token_free]) so every matmul contracts along the SBUF
partition dim with zero on-device transposes.  The host pre-transposes
q/y once (cheap numpy) when building the per-core input maps.

scoresT [m, n] = (k_projT as lhsT).T-free @ q_projT   -> partition = keys m
exp runs on ScalarE with the 1/sqrt(E) folded into the activation scale;
no max-subtraction is needed (weights are scale 0.02 -> |score| < ~3).
The softmax denominator comes from an extra 1-column matmul against a
ones vector that rides on the same loaded weights (eT block) as the
out-matmuls; the output block is then scaled by the reciprocal.

Matmul dtypes: projections in fp32r (full-rate on TRN2 for free-dim >=
256, ~tf32 accuracy, zero cast cost from the fp32 inputs); attention
matmuls in bf16 (projection outputs are rounded to bf16 on the
PSUM->SBUF copy, halving SBUF so everything stays resident).
"""

import numpy as np
from contextlib import ExitStack

import concourse.bass as bass
import concourse.tile as tile
from concourse import bacc, mybir
from concourse.bass_utils import run_bass_kernel_spmd

P = 128
F32 = mybir.dt.float32
F32R = mybir.dt.float32r
BF16 = mybir.dt.bfloat16

# Problem shapes (hardcoded per contract)
B = 8
NQ = 2048
NK = 2048
D = 1024   # in_q_dim == in_dim
E = 1024   # hid_q == out_dim
F = 1024   # out_dim (v)


def build_program(
    nq=NQ, nk=NK, d=D, e=E, f=F,
    nblk=512,          # query block (columns of q_projT processed per round)
    mblk=512,          # key block for the k-projection phase
    proj_dtype="f32r",  # matmul dtype for the three projections
):
    """Build the single-core Bass program (same program runs SPMD on all cores)."""
    nc = bacc.Bacc(trn_type="TRN2")

    DC = d // P            # contraction chunks for the projections
    EC = e // P
    MC = nk // P           # key chunks
    MB = nk // mblk
    NB = nq // nblk
    NSUB = nblk // P
    FCH = (f + 511) // 512  # 512-wide chunks of the value dim
    fch = [min(512, f - 512 * j) for j in range(FCH)]
    sch = min(512, nblk)   # scores free dim per matmul == nblk (<=512)
    assert nblk <= 512 and mblk <= 512

    pf = F32R if proj_dtype == "f32r" else F32
    qT = nc.dram_tensor("qT", [d, nq], pf, kind="ExternalInput").ap()
    yT = nc.dram_tensor("yT", [d, nk], pf, kind="ExternalInput").ap()
    Wq = nc.dram_tensor("Wq", [d, e], pf, kind="ExternalInput").ap()
    Wk = nc.dram_tensor("Wk", [d, e], pf, kind="ExternalInput").ap()
    Wv = nc.dram_tensor("Wv", [e, f], F32, kind="ExternalInput").ap()
    out = nc.dram_tensor("out", [nq, f], F32, kind="ExternalOutput").ap()

    qT_v = qT.rearrange("(c p) n -> p c n", p=P)     # [P, DC, nq]
    yT_v = yT.rearrange("(c p) n -> p c n", p=P)     # [P, DC, nk]
    Wq_v = Wq.rearrange("(c p) e -> p c e", p=P)     # [P, DC, e]
    Wk_v = Wk.rearrange("(c p) e -> p c e", p=P)
    Wv_v = Wv.rearrange("(c p) f -> p c f", p=P)     # [P, EC, f]
    out_v = out.rearrange("(b p) f -> b p f", p=P)   # [nq//P, P, f]

    def pdt(ap):
        return ap

    with tile.TileContext(nc) as tc, ExitStack() as ctx:
        consts = ctx.enter_context(tc.tile_pool(name="consts", bufs=1))
        staging = ctx.enter_context(tc.tile_pool(name="staging", bufs=2))
        kproj_pool = ctx.enter_context(tc.tile_pool(name="kproj", bufs=1))
        v_pool = ctx.enter_context(tc.tile_pool(name="vproj", bufs=1))
        wq_pool = ctx.enter_context(tc.tile_pool(name="wq", bufs=1))
        psum_a = ctx.enter_context(
            tc.tile_pool(name="psum_a", bufs=3, space="PSUM"))

        ones_bf = consts.tile([P, 1], BF16)
        nc.vector.memset(ones_bf, 1.0)
        zbias = consts.tile([P, 1], F32)
        nc.vector.memset(zbias, 0.0)

        kprojT = kproj_pool.tile([P, EC, nk], BF16)   # [e_part, e_chunk, m]
        v_sb = v_pool.tile([P, MC, f], BF16)          # [m_part, m_chunk, f]
        wq_sb = wq_pool.tile([P, DC, e], pf)

        # ---- Phase 1+2: k-projection, then v-projection (transient weights) --
        with tc.tile_pool(name="wk", bufs=1) as wk_pool, \
             tc.tile_pool(name="wvbf", bufs=1) as wv_pool:
            # Startup critical path: interleave the first yT block (sync
            # queue) with Wk (scalar queue) in d-chunk pieces so the first
            # matmul starts after ~1.5MB instead of 10MB of DMA.
            wk_sb = wk_pool.tile([P, DC, e], pf)
            yt0 = staging.tile([P, DC, mblk], pf, tag="stage", name="yt0")
            DSP = max(1, DC // 4)
            for c in range(0, DC, DSP):
                nc.sync.dma_start(yt0[:, c:c + DSP, :],
                                  yT_v[:, c:c + DSP, 0:mblk])
                nc.sync.dma_start(wk_sb[:, c:c + DSP, :],
                                   Wk_v[:, c:c + DSP, :])

            # Wv: load fp32 through staging, round to bf16 on DVE
            wv_bf = wv_pool.tile([P, EC, f], BF16)
            for j in range(FCH):
                st = staging.tile([P, DC, mblk], F32, tag="stage", name="st")
                nc.sync.dma_start(st[:, :, :fch[j]],
                                  Wv_v[:, :, 512 * j: 512 * j + fch[j]])
                nc.vector.tensor_copy(wv_bf[:, :, 512 * j: 512 * j + fch[j]],
                                      st[:, :, :fch[j]])

            # k_projT[e, m] = sum_d Wk[d, e].T @ yT[d, m]
            for mb in range(MB):
                if mb == 0:
                    yt = yt0
                else:
                    yt = staging.tile([P, DC, mblk], pf, tag="stage", name="yt")
                    nc.sync.dma_start(yt, yT_v[:, :, mb * mblk:(mb + 1) * mblk])
                for ei in range(EC):
                    ps = psum_a.tile([P, 512], F32, tag="psa", name="psa")[:, :mblk]
                    for di in range(DC):
                        nc.tensor.matmul(
                            ps,
                            lhsT=pdt(wk_sb[:, di, ei * P:(ei + 1) * P]),
                            rhs=pdt(yt[:, di, :]),
                            start=(di == 0), stop=(di == DC - 1))
                    nc.vector.tensor_copy(
                        kprojT[:, ei, mb * mblk:(mb + 1) * mblk], ps)

            # prefetch Wq during the (DMA-free) v phase (scalar queue)
            nc.sync.dma_start(wq_sb, Wq_v)

            # v[m, f] = sum_e k_projT[e, m].T @ Wv[e, f]   (bf16)
            for mi in range(MC):
                for j in range(FCH):
                    ps = psum_a.tile([P, 512], F32, tag="psa", name="psa")[:, :fch[j]]
                    for ei in range(EC):
                        nc.tensor.matmul(
                            ps,
                            lhsT=kprojT[:, ei, mi * P:(mi + 1) * P],
                            rhs=wv_bf[:, ei, 512 * j: 512 * j + fch[j]],
                            start=(ei == 0), stop=(ei == EC - 1))
                    nc.vector.tensor_copy(v_sb[:, mi, 512 * j: 512 * j + fch[j]], ps)

        # ---- Phase 3: attention, blocked over queries ----
        qproj_pool = ctx.enter_context(tc.tile_pool(name="qproj", bufs=2))
        eT_pool = ctx.enter_context(tc.tile_pool(name="eT", bufs=2))
        out_pool = ctx.enter_context(tc.tile_pool(name="outsb", bufs=2))
        small = ctx.enter_context(tc.tile_pool(name="small", bufs=6))
        psum_o = ctx.enter_context(
            tc.tile_pool(name="psum_o", bufs=4, space="PSUM"))
        psum_s = ctx.enter_context(
            tc.tile_pool(name="psum_s", bufs=1, space="PSUM"))

        for nb in range(NB):
            qt = staging.tile([P, DC, nblk], pf, tag="stage")
            nc.sync.dma_start(qt, qT_v[:, :, nb * nblk:(nb + 1) * nblk])

            # q_projT[e, n_blk]  (bf16)
            qp = qproj_pool.tile([P, EC, nblk], BF16)
            for ei in range(EC):
                ps = psum_a.tile([P, 512], F32, tag="psa", name="psa")[:, :nblk]
                for di in range(DC):
                    nc.tensor.matmul(
                        ps,
                        lhsT=pdt(wq_sb[:, di, ei * P:(ei + 1) * P]),
                        rhs=pdt(qt[:, di, :]),
                        start=(di == 0), stop=(di == DC - 1))
                nc.vector.tensor_copy(qp[:, ei, :], ps)

            # eT[m, n_blk] = exp(scoresT / sqrt(E))
            eT = eT_pool.tile([P, MC, nblk], BF16)
            for mi in range(MC):
                ps = psum_a.tile([P, 512], F32, tag="psa", name="psa")[:, :sch]
                for ei in range(EC):
                    nc.tensor.matmul(
                        ps,
                        lhsT=kprojT[:, ei, mi * P:(mi + 1) * P],
                        rhs=qp[:, ei, :],
                        start=(ei == 0), stop=(ei == EC - 1))
                nc.scalar.activation(
                    eT[:, mi, :], ps,
                    mybir.ActivationFunctionType.Exp,
                    bias=zbias, scale=1.0 / float(np.sqrt(e)))

            # out[n, f] = (eT.T @ v) / (eT.T @ 1)
            for ns in range(NSUB):
                pos = [psum_o.tile([P, 512], F32, tag="pso", name="pso")[:, :fch[j]]
                       for j in range(FCH)]
                pss = psum_s.tile([P, 1], F32, tag="pss", name="pss")
                for mi in range(MC):
                    lhsT_e = eT[:, mi, ns * P:(ns + 1) * P]
                    for j in range(FCH):
                        nc.tensor.matmul(
                            pos[j], lhsT=lhsT_e,
                            rhs=v_sb[:, mi, 512 * j: 512 * j + fch[j]],
                            start=(mi == 0), stop=(mi == MC - 1))
                    nc.tensor.matmul(
                        pss, lhsT=lhsT_e, rhs=ones_bf,
                        start=(mi == 0), stop=(mi == MC - 1))
                rec = small.tile([P, 1], F32)
                nc.vector.reciprocal(rec, pss)
                ob = out_pool.tile([P, f], F32)
                for j in range(FCH):
                    nc.vector.tensor_scalar_mul(
                        ob[:, 512 * j: 512 * j + fch[j]], pos[j], rec)
                    nc.sync.dma_start(
                        out_v[nb * NSUB + ns][:, 512 * j: 512 * j + fch[j]],
                        ob[:, 512 * j: 512 * j + fch[j]])

    nc.compile()
    return nc


_CACHE = {}


def kernel(q, y, Wq, Wk, Wv):
    q = np.asarray(q, dtype=np.float32)
    y = np.asarray(y, dtype=np.float32)
    Wq = np.ascontiguousarray(np.asarray(Wq, dtype=np.float32))
    Wk = np.ascontiguousarray(np.asarray(Wk, dtype=np.float32))
    Wv = np.ascontiguousarray(np.asarray(Wv, dtype=np.float32))

    if "nc" not in _CACHE:
        _CACHE["nc"] = build_program()
    nc = _CACHE["nc"]

    in_maps = []
    for b in range(B):
        in_maps.append({
            "qT": np.ascontiguousarray(q[b].T),
            "yT": np.ascontiguousarray(y[b].T),
            "Wq": Wq, "Wk": Wk, "Wv": Wv,
        })
    res = run_bass_kernel_spmd(nc, in_maps, core_ids=list(range(B)))
    return np.stack([res.results[b]["out"] for b in range(B)], axis=0)



# revision 2
# speedup vs baseline: 1.4549x; 1.4549x over previous
"""Cross-attention kernel for Trainium2, 8 NeuronCores, data-parallel over batch.

Algebraic restructuring (weights folded on host, in fp64 -- free):
    Wqk = Wq @ Wk.T        [d, d]
    Wkv = Wk @ Wv          [d, f]
so that per batch b (one batch per core):
    qh     = q[b] @ Wqk            [Nq, d]    (was q_proj, now in y-feature basis)
    scores = qh @ y[b].T / 32      [Nq, Nk]   (== q_proj @ k_proj.T / sqrt(E))
    v      = y[b] @ Wkv            [Nk, F]    (== k_proj @ Wv)
    out    = softmax(scores) @ v
This removes the entire k-projection (256 matmuls/core) from the device.

Device layout: activations are feature-major ([feature_part, token_free]) so
every matmul contracts along the SBUF partition dim with zero on-device
transposes. The host pre-transposes q/y and pre-casts everything to fp16
(same PE rate as bf16, 4x less rounding noise, half the DMA bytes).

scoresT [m, n] = (yT as lhsT).T @ qhT  -> partition = keys m
exp on ScalarE with 1/sqrt(E) folded into the activation scale (scores are
small, |s| < ~3, so no max-subtraction needed).
Softmax denominator: DVE sums eT over the 16 key-chunks -> esum [m_part, n],
then one 1-column ones-matmul per 128-query group gives sum over partitions;
out block is scaled by the reciprocal on DVE.
"""

import numpy as np
from contextlib import ExitStack

import concourse.bass as bass
import concourse.tile as tile
from concourse import bacc, mybir
from concourse.bass_utils import run_bass_kernel_spmd

P = 128
F32 = mybir.dt.float32
F16 = mybir.dt.float16

# Problem shapes (hardcoded per contract)
B = 8
NQ = 2048
NK = 2048
D = 1024   # in_q_dim == in_dim (folded: qh lives in the y-feature basis)
F = 1024   # out_dim (v)


def build_program(nq=NQ, nk=NK, d=D, f=F, nblk=512):
    """Single-core Bass program (same program runs SPMD on all cores)."""
    nc = bacc.Bacc(trn_type="TRN2")

    DC = d // P            # feature chunks (contraction for qh/scores/v)
    MC = nk // P           # key chunks (contraction for out)
    MB = nk // 512         # 512-wide key blocks for the v phase / yT DMA
    NB = nq // nblk        # query blocks
    NSUB = nblk // P       # 128-query subblocks per block
    FCH = f // 512         # 512-wide chunks of the value dim
    assert nblk <= 512

    qT = nc.dram_tensor("qT", [d, nq], F16, kind="ExternalInput").ap()
    yT = nc.dram_tensor("yT", [d, nk], F16, kind="ExternalInput").ap()
    Wqk = nc.dram_tensor("Wqk", [d, d], F16, kind="ExternalInput").ap()
    Wkv = nc.dram_tensor("Wkv", [d, f], F16, kind="ExternalInput").ap()
    out = nc.dram_tensor("out", [nq, f], F32, kind="ExternalOutput").ap()

    qT_v = qT.rearrange("(c p) n -> p c n", p=P)     # [P, DC, nq]
    yT_v = yT.rearrange("(c p) n -> p c n", p=P)     # [P, DC, nk]
    Wqk_v = Wqk.rearrange("(c p) e -> p c e", p=P)   # [P, DC, d]
    Wkv_v = Wkv.rearrange("(c p) f -> p c f", p=P)   # [P, DC, f]
    out_v = out.rearrange("(b p) f -> b p f", p=P)   # [nq//P, P, f]

    with tile.TileContext(nc) as tc, ExitStack() as ctx:
        consts = ctx.enter_context(tc.tile_pool(name="consts", bufs=1))
        y_pool = ctx.enter_context(tc.tile_pool(name="ysb", bufs=1))
        v_pool = ctx.enter_context(tc.tile_pool(name="vproj", bufs=1))
        wqk_pool = ctx.enter_context(tc.tile_pool(name="wqk", bufs=1))
        staging = ctx.enter_context(tc.tile_pool(name="staging", bufs=2))
        psum_a = ctx.enter_context(
            tc.tile_pool(name="psum_a", bufs=3, space="PSUM"))

        ones16 = consts.tile([P, 1], F16)
        nc.vector.memset(ones16, 1.0)
        zbias = consts.tile([P, 1], F32)
        nc.vector.memset(zbias, 0.0)

        y_sb = y_pool.tile([P, DC, nk], F16)     # full yT, resident
        v_sb = v_pool.tile([P, MC, f], F16)      # [m_part, m_chunk, f]
        wqk_sb = wqk_pool.tile([P, DC, d], F16)

        # ---- Phase 1: v = y @ Wkv (transient Wkv weights) ----
        with tc.tile_pool(name="wkv", bufs=1) as wkv_pool:
            wkv_sb = wkv_pool.tile([P, DC, f], F16)
            # interleave first yT block and Wkv in d-chunk pieces so the
            # first matmul starts after ~0.5MB instead of 4MB of DMA
            DSP = 2
            for c in range(0, DC, DSP):
                nc.sync.dma_start(wkv_sb[:, c:c + DSP, :],
                                  Wkv_v[:, c:c + DSP, :])
                nc.sync.dma_start(y_sb[:, c:c + DSP, 0:512],
                                  yT_v[:, c:c + DSP, 0:512])

            for mb in range(MB):
                if mb > 0:
                    nc.sync.dma_start(y_sb[:, :, mb * 512:(mb + 1) * 512],
                                      yT_v[:, :, mb * 512:(mb + 1) * 512])
                if mb == 1:
                    # prefetch Wqk + first q block behind the yT stream
                    nc.sync.dma_start(wqk_sb, Wqk_v)
                for r in range(512 // P):
                    mi = mb * (512 // P) + r
                    for j in range(FCH):
                        ps = psum_a.tile([P, 512], F32, tag="psa", name="psa")
                        for di in range(DC):
                            nc.tensor.matmul(
                                ps,
                                lhsT=y_sb[:, di, mi * P:(mi + 1) * P],
                                rhs=wkv_sb[:, di, j * 512:(j + 1) * 512],
                                start=(di == 0), stop=(di == DC - 1))
                        nc.vector.tensor_copy(v_sb[:, mi, j * 512:(j + 1) * 512], ps)

        # ---- Phase 2: attention, blocked over queries ----
        qh_pool = ctx.enter_context(tc.tile_pool(name="qh", bufs=2))
        eT_pool = ctx.enter_context(tc.tile_pool(name="eT", bufs=2))
        esum_pool = ctx.enter_context(tc.tile_pool(name="esum", bufs=2))
        out_pool = ctx.enter_context(tc.tile_pool(name="outsb", bufs=2))
        small = ctx.enter_context(tc.tile_pool(name="small", bufs=8))
        psum_o = ctx.enter_context(
            tc.tile_pool(name="psum_o", bufs=4, space="PSUM"))
        psum_s = ctx.enter_context(
            tc.tile_pool(name="psum_s", bufs=1, space="PSUM"))

        for nb in range(NB):
            qt = staging.tile([P, DC, nblk], F16, tag="stage")
            nc.sync.dma_start(qt, qT_v[:, :, nb * nblk:(nb + 1) * nblk])

            # qhT[d2, n_blk] (fp16)
            qh = qh_pool.tile([P, DC, nblk], F16)
            for ei in range(DC):
                ps = psum_a.tile([P, 512], F32, tag="psa", name="psa")[:, :nblk]
                for di in range(DC):
                    nc.tensor.matmul(
                        ps,
                        lhsT=wqk_sb[:, di, ei * P:(ei + 1) * P],
                        rhs=qt[:, di, :],
                        start=(di == 0), stop=(di == DC - 1))
                nc.vector.tensor_copy(qh[:, ei, :], ps)

            # eT[m, n_blk] = exp(scoresT / sqrt(E))
            eT = eT_pool.tile([P, MC, nblk], F16)
            for mi in range(MC):
                ps = psum_a.tile([P, 512], F32, tag="psa", name="psa")[:, :nblk]
                for di in range(DC):
                    nc.tensor.matmul(
                        ps,
                        lhsT=y_sb[:, di, mi * P:(mi + 1) * P],
                        rhs=qh[:, di, :],
                        start=(di == 0), stop=(di == DC - 1))
                nc.scalar.activation(
                    eT[:, mi, :], ps,
                    mybir.ActivationFunctionType.Exp,
                    bias=zbias, scale=1.0 / float(np.sqrt(d)))

            # esum[m, n] = sum_mi eT  (DVE, overlaps the out matmuls)
            esum32 = esum_pool.tile([P, nblk], F32, tag="es32")
            esum16 = esum_pool.tile([P, nblk], F16, tag="es16")
            for ns in range(NSUB):
                sl = slice(ns * P, (ns + 1) * P)
                nc.vector.tensor_reduce(
                    esum32[:, sl],
                    eT[:, :, sl].rearrange("p c n -> p n c"),
                    axis=mybir.AxisListType.X, op=mybir.AluOpType.add)
                nc.vector.tensor_copy(esum16[:, sl], esum32[:, sl])

            # out[n, f] = (eT.T @ v) / (eT.T @ 1)
            for ns in range(NSUB):
                pos = [psum_o.tile([P, 512], F32, tag="pso", name="pso")
                       for j in range(FCH)]
                for mi in range(MC):
                    lhsT_e = eT[:, mi, ns * P:(ns + 1) * P]
                    for j in range(FCH):
                        nc.tensor.matmul(
                            pos[j], lhsT=lhsT_e,
                            rhs=v_sb[:, mi, j * 512:(j + 1) * 512],
                            start=(mi == 0), stop=(mi == MC - 1))
                pss = psum_s.tile([P, 1], F32, tag="pss", name="pss")
                nc.tensor.matmul(pss, lhsT=esum16[:, ns * P:(ns + 1) * P],
                                 rhs=ones16, start=True, stop=True)
                rec = small.tile([P, 1], F32)
                nc.vector.reciprocal(rec, pss)
                ob = out_pool.tile([P, f], F32)
                for j in range(FCH):
                    nc.vector.tensor_scalar_mul(
                        ob[:, j * 512:(j + 1) * 512], pos[j], rec)
                    nc.sync.dma_start(
                        out_v[nb * NSUB + ns][:, j * 512:(j + 1) * 512],
                        ob[:, j * 512:(j + 1) * 512])

    nc.compile()
    return nc


_CACHE = {}


def make_in_maps(q, y, Wq, Wk, Wv):
    """Host-side prep: fold weights (fp64), transpose + cast to fp16."""
    Wqk = (np.asarray(Wq, np.float64) @ np.asarray(Wk, np.float64).T)
    Wkv = (np.asarray(Wk, np.float64) @ np.asarray(Wv, np.float64))
    Wqk16 = np.ascontiguousarray(Wqk, np.float16)
    Wkv16 = np.ascontiguousarray(Wkv, np.float16)
    q = np.asarray(q)
    y = np.asarray(y)
    in_maps = []
    for b in range(B):
        in_maps.append({
            "qT": np.ascontiguousarray(q[b].T, np.float16),
            "yT": np.ascontiguousarray(y[b].T, np.float16),
            "Wqk": Wqk16, "Wkv": Wkv16,
        })
    return in_maps


def kernel(q, y, Wq, Wk, Wv):
    if "nc" not in _CACHE:
        _CACHE["nc"] = build_program()
    nc = _CACHE["nc"]
    in_maps = make_in_maps(q, y, Wq, Wk, Wv)
    res = run_bass_kernel_spmd(nc, in_maps, core_ids=list(range(B)))
    return np.stack([res.results[b]["out"] for b in range(B)], axis=0)


# revision 3
# speedup vs baseline: 1.7822x; 1.2250x over previous
"""Cross-attention kernel for Trainium2, 8 NeuronCores, data-parallel over batch.

Algebraic restructuring (weights folded on host, in fp64 -- free):
    Wqk = Wq @ Wk.T        [d, d]
    Wkv = Wk @ Wv          [d, f]
so that per batch b (one batch per core):
    qh     = q[b] @ Wqk            [Nq, d]    (q-projection in the y-feature basis)
    scores = qh @ y[b].T / 32      [Nq, Nk]   (== q_proj @ k_proj.T / sqrt(E))
    v      = y[b] @ Wkv            [Nk, F]    (== k_proj @ Wv)
    out    = softmax(scores) @ v
This removes the entire k-projection (256 matmuls/core) from the device.

Device layout: activations are feature-major ([feature_part, token_free]) so
every matmul contracts along the SBUF partition dim with zero on-device
transposes. The host pre-transposes q/y and pre-casts to fp16 (same PE rate
as bf16, 4x less rounding noise, half the DMA bytes).

The scores matmul (the only place where fp8 noise is attenuated enough --
score std is ~0.43, and softmax turns absolute score error into relative
attn error) runs in fp8e4 with perf_mode=DoubleRow: 2 fp8 weights per PE
cell, contraction 256 per matmul, ~2x FLOP rate. qh is quantized to fp8
(x8 scale, folded into the exp) on the ScalarE PSUM->SBUF copy; y arrives
fp8 from the host. Simulated end-to-end error 0.0163 vs the 2e-2 gate
(the value path stays fp16: fp8 there puts ~3% noise straight on out).

scoresT [m, n] = (y8 as lhsT).T @ qh8  -> partition = keys m
exp on ScalarE with 1/(sqrt(E)*8) folded into the activation scale (scores
are small, |s| < ~3, so no max-subtraction needed).
Softmax denominator: eT chunks are accumulated into esum[m_part, n] on DVE
as each exp lands (so it's ready before the out matmuls drain), then one
1-column ones-matmul per 128-query group reduces over partitions; the out
block is scaled by the reciprocal on DVE.
"""

import numpy as np
import ml_dtypes
from contextlib import ExitStack

import concourse.bass as bass
import concourse.tile as tile
from concourse import bacc, mybir
from concourse.bass_utils import run_bass_kernel_spmd

P = 128
F32 = mybir.dt.float32
F16 = mybir.dt.float16
FP8 = mybir.dt.float8e4

# Problem shapes (hardcoded per contract)
B = 8
NQ = 2048
NK = 2048
D = 1024   # in_q_dim == in_dim (folded: qh lives in the y-feature basis)
F = 1024   # out_dim (v)

USE_FP8_SCORES = True
QH8_SCALE = 8.0


def build_program(nq=NQ, nk=NK, d=D, f=F, nblk=512, fp8_scores=USE_FP8_SCORES):
    """Single-core Bass program (same program runs SPMD on all cores)."""
    nc = bacc.Bacc(trn_type="TRN2")

    DC = d // P            # feature chunks (contraction for qh/scores/v)
    MC = nk // P           # key chunks (contraction for out)
    MB = nk // 512         # 512-wide key blocks for the v phase / yT DMA
    NB = nq // nblk        # query blocks
    NSUB = nblk // P       # 128-query subblocks per block
    FCH = f // 512         # 512-wide chunks of the value dim
    assert nblk <= 512

    qT = nc.dram_tensor("qT", [d, nq], F16, kind="ExternalInput").ap()
    yT = nc.dram_tensor("yT", [d, nk], F16, kind="ExternalInput").ap()
    Wqk = nc.dram_tensor("Wqk", [d, d], F16, kind="ExternalInput").ap()
    Wkv = nc.dram_tensor("Wkv", [d, f], F16, kind="ExternalInput").ap()
    if fp8_scores:
        yT8 = nc.dram_tensor("yT8", [d, nk], FP8, kind="ExternalInput").ap()
        yT8_v = yT8.rearrange("(c p) n -> p c n", p=P)
    out = nc.dram_tensor("out", [nq, f], F32, kind="ExternalOutput").ap()

    qT_v = qT.rearrange("(c p) n -> p c n", p=P)     # [P, DC, nq]
    yT_v = yT.rearrange("(c p) n -> p c n", p=P)     # [P, DC, nk]
    Wqk_v = Wqk.rearrange("(c p) e -> p c e", p=P)   # [P, DC, d]
    Wkv_v = Wkv.rearrange("(c p) f -> p c f", p=P)   # [P, DC, f]
    out_v = out.rearrange("(b p) f -> b p f", p=P)   # [nq//P, P, f]

    with tile.TileContext(nc) as tc, ExitStack() as ctx:
        consts = ctx.enter_context(tc.tile_pool(name="consts", bufs=1))
        y_pool = ctx.enter_context(tc.tile_pool(name="ysb", bufs=1))
        v_pool = ctx.enter_context(tc.tile_pool(name="vproj", bufs=1))
        wqk_pool = ctx.enter_context(tc.tile_pool(name="wqk", bufs=1))
        staging = ctx.enter_context(tc.tile_pool(name="staging", bufs=2))
        psum_a = ctx.enter_context(
            tc.tile_pool(name="psum_a", bufs=3, space="PSUM"))

        ones16 = consts.tile([P, 1], F16)
        nc.vector.memset(ones16, 1.0)
        zbias = consts.tile([P, 1], F32)
        nc.vector.memset(zbias, 0.0)

        y_sb = y_pool.tile([P, DC, nk], F16)     # full yT, resident
        v_sb = v_pool.tile([P, MC, f], F16)      # [m_part, m_chunk, f]
        wqk_sb = wqk_pool.tile([P, DC, d], F16)
        if fp8_scores:
            y8_sb = y_pool.tile([P, DC, nk], FP8)

        # ---- Phase 1: v = y @ Wkv (transient Wkv weights) ----
        # weights stream on the scalar-engine DMA queue, activations on the
        # sync queue, so the first matmul's deps land after ~0.75MB not 3MB
        with tc.tile_pool(name="wkv", bufs=1) as wkv_pool:
            wkv_sb = wkv_pool.tile([P, DC, f], F16)
            DSP = 2
            for c in range(0, DC, DSP):
                nc.scalar.dma_start(wkv_sb[:, c:c + DSP, :],
                                    Wkv_v[:, c:c + DSP, :])
                nc.sync.dma_start(y_sb[:, c:c + DSP, 0:512],
                                  yT_v[:, c:c + DSP, 0:512])

            for mb in range(MB):
                if mb > 0:
                    nc.sync.dma_start(y_sb[:, :, mb * 512:(mb + 1) * 512],
                                      yT_v[:, :, mb * 512:(mb + 1) * 512])
                if mb == 1:
                    nc.scalar.dma_start(wqk_sb, Wqk_v)
                    if fp8_scores:
                        nc.scalar.dma_start(y8_sb, yT8_v)
                for r in range(512 // P):
                    mi = mb * (512 // P) + r
                    for j in range(FCH):
                        ps = psum_a.tile([P, 512], F32, tag="psa", name="psa")
                        for di in range(DC):
                            nc.tensor.matmul(
                                ps,
                                lhsT=y_sb[:, di, mi * P:(mi + 1) * P],
                                rhs=wkv_sb[:, di, j * 512:(j + 1) * 512],
                                start=(di == 0), stop=(di == DC - 1))
                        nc.vector.tensor_copy(v_sb[:, mi, j * 512:(j + 1) * 512], ps)

        # ---- Phase 2: attention, blocked over queries ----
        qh_pool = ctx.enter_context(tc.tile_pool(name="qh", bufs=2))
        eT_pool = ctx.enter_context(tc.tile_pool(name="eT", bufs=2))
        esum_pool = ctx.enter_context(tc.tile_pool(name="esum", bufs=2))
        out_pool = ctx.enter_context(tc.tile_pool(name="outsb", bufs=2))
        small = ctx.enter_context(tc.tile_pool(name="small", bufs=8))
        psum_o = ctx.enter_context(
            tc.tile_pool(name="psum_o", bufs=4, space="PSUM"))
        psum_s = ctx.enter_context(
            tc.tile_pool(name="psum_s", bufs=1, space="PSUM"))

        exp_scale = 1.0 / float(np.sqrt(d))
        if fp8_scores:
            exp_scale /= QH8_SCALE

        for nb in range(NB):
            qt = staging.tile([P, DC, nblk], F16, tag="stage")
            nc.sync.dma_start(qt, qT_v[:, :, nb * nblk:(nb + 1) * nblk])

            # qhT[d2, n_blk]
            qh = qh_pool.tile([P, DC, nblk], FP8 if fp8_scores else F16)
            for ei in range(DC):
                ps = psum_a.tile([P, 512], F32, tag="psa", name="psa")[:, :nblk]
                for di in range(DC):
                    nc.tensor.matmul(
                        ps,
                        lhsT=wqk_sb[:, di, ei * P:(ei + 1) * P],
                        rhs=qt[:, di, :],
                        start=(di == 0), stop=(di == DC - 1))
                if fp8_scores:
                    # quantize to fp8 on ScalarE; x8 scale folded into exp
                    nc.scalar.mul(qh[:, ei, :], ps, QH8_SCALE)
                else:
                    nc.vector.tensor_copy(qh[:, ei, :], ps)

            # eT[m, n_blk] = exp(scoresT / sqrt(E)); esum accumulates on DVE
            eT = eT_pool.tile([P, MC, nblk], F16)
            esum32 = esum_pool.tile([P, nblk], F32, tag="es32")
            esum16 = esum_pool.tile([P, nblk], F16, tag="es16")
            for mi in range(MC):
                ps = psum_a.tile([P, 512], F32, tag="psa", name="psa")[:, :nblk]
                if fp8_scores:
                    for dp in range(DC // 2):
                        nc.tensor.matmul(
                            ps,
                            lhsT=y8_sb[:, 2 * dp:2 * dp + 2, mi * P:(mi + 1) * P],
                            rhs=qh[:, 2 * dp:2 * dp + 2, :],
                            start=(dp == 0), stop=(dp == DC // 2 - 1),
                            perf_mode=mybir.MatmulPerfMode.DoubleRow)
                else:
                    for di in range(DC):
                        nc.tensor.matmul(
                            ps,
                            lhsT=y_sb[:, di, mi * P:(mi + 1) * P],
                            rhs=qh[:, di, :],
                            start=(di == 0), stop=(di == DC - 1))
                nc.scalar.activation(
                    eT[:, mi, :], ps,
                    mybir.ActivationFunctionType.Exp,
                    bias=zbias, scale=exp_scale)
                if mi == 0:
                    nc.vector.tensor_copy(esum32, eT[:, 0, :])
                else:
                    nc.vector.scalar_tensor_tensor(
                        esum32, eT[:, mi, :], 1.0, esum32,
                        op0=mybir.AluOpType.mult, op1=mybir.AluOpType.add)
            nc.vector.tensor_copy(esum16, esum32)

            # out[n, f] = (eT.T @ v) / (eT.T @ 1)
            for ns in range(NSUB):
                pos = [psum_o.tile([P, 512], F32, tag="pso", name="pso")
                       for j in range(FCH)]
                for mi in range(MC):
                    lhsT_e = eT[:, mi, ns * P:(ns + 1) * P]
                    for j in range(FCH):
                        nc.tensor.matmul(
                            pos[j], lhsT=lhsT_e,
                            rhs=v_sb[:, mi, j * 512:(j + 1) * 512],
                            start=(mi == 0), stop=(mi == MC - 1))
                pss = psum_s.tile([P, 1], F32, tag="pss", name="pss")
                nc.tensor.matmul(pss, lhsT=esum16[:, ns * P:(ns + 1) * P],
                                 rhs=ones16, start=True, stop=True)
                rec = small.tile([P, 1], F32)
                nc.vector.reciprocal(rec, pss)
                ob = out_pool.tile([P, f], F32)
                for j in range(FCH):
                    nc.vector.tensor_scalar_mul(
                        ob[:, j * 512:(j + 1) * 512], pos[j], rec)
                    nc.sync.dma_start(
                        out_v[nb * NSUB + ns][:, j * 512:(j + 1) * 512],
                        ob[:, j * 512:(j + 1) * 512])

    nc.compile()
    return nc


_CACHE = {}


def make_in_maps(q, y, Wq, Wk, Wv):
    """Host-side prep: fold weights (fp64), transpose + cast."""
    Wqk = (np.asarray(Wq, np.float64) @ np.asarray(Wk, np.float64).T)
    Wkv = (np.asarray(Wk, np.float64) @ np.asarray(Wv, np.float64))
    Wqk16 = np.ascontiguousarray(Wqk, np.float16)
    Wkv16 = np.ascontiguousarray(Wkv, np.float16)
    q = np.asarray(q)
    y = np.asarray(y)
    in_maps = []
    for b in range(B):
        yTb = np.ascontiguousarray(y[b].T, np.float16)
        m = {
            "qT": np.ascontiguousarray(q[b].T, np.float16),
            "yT": yTb,
            "Wqk": Wqk16, "Wkv": Wkv16,
        }
        if USE_FP8_SCORES:
            m["yT8"] = np.clip(y[b].T, -240, 240).astype(ml_dtypes.float8_e4m3)
        in_maps.append(m)
    return in_maps


def kernel(q, y, Wq, Wk, Wv):
    if "nc" not in _CACHE:
        _CACHE["nc"] = build_program()
    nc = _CACHE["nc"]
    in_maps = make_in_maps(q, y, Wq, Wk, Wv)
    res = run_bass_kernel_spmd(nc, in_maps, core_ids=list(range(B)))
    return np.stack([res.results[b]["out"] for b in range(B)], axis=0)
